# revision 8
# baseline (speedup 1.0000x reference)
"""Distance-aware transformer encoder layer on 8 Trainium2 NeuronCores.

Sharding: core c handles batch b = c//2 and query-half qh = c%2 (512 query
rows). K/V are computed per-core for the full 1024-key sequence of its batch
(duplicated across the core pair — cheaper than a collective). Everything
else (scores, softmax, out-proj, LayerNorms, FFN) is perfectly sharded by
query rows. No collectives.

Precision: bf16 operands for every matmul (fp32 PSUM accumulation); fp32
residual path (src_q, xpre, x, ypre) and LayerNorm statistics. The distance
bias is folded in exp-space: host ships P^T = (d+1e-9)^(-|dist_scale|) as
bf16 and the kernel computes E = exp(scores) * P on Scalar+Vector, which
removes the per-head identity-matmul bias injection of the earlier version.
Softmax normalization broadcasts 1/rowsum via a 1-row PE matmul instead of
a DRAM round-trip. W2 streams through SBUF once (8 PSUM banks accumulate
all 4 query tiles x 2 output halves simultaneously).
"""

import numpy as np
import ml_dtypes

import bass_rust
import concourse.bass as bass
import concourse.tile as tile
import concourse.mybir as mybir
from concourse.bass import AP
from concourse.bass_utils import run_bass_kernel_spmd

B, S, D, H, DFF, HD = 4, 1024, 1024, 16, 4096, 64
SQ = 512          # query rows per core
NCORES = 8
EPS = 1e-5
F32 = mybir.dt.float32
F32R = mybir.dt.float32r
BF16 = mybir.dt.bfloat16
FT = mybir.ActivationFunctionType
ALU = mybir.AluOpType

_nop_ctr = [0]


def _legalize_waits(nc):
    """walrus codegen in this toolchain accepts only one sync-wait per
    instruction; split extras onto same-engine NoOps inserted before."""
    n_fixed = 0
    for f in nc.m.functions:
        for bb in f.blocks:
            insts = bb.instructions
            i = 0
            while i < len(insts):
                inst = insts[i]
                si = inst.sync_info
                waits = list(si.on_wait) if si is not None and si.on_wait else []
                if len(waits) > 1:
                    keep = waits[-1]
                    for w in waits[:-1]:
                        n = bass_rust.InstNoOp(
                            name=f"waitsplit-nop-{_nop_ctr[0]}", ins=[], outs=[]
                        )
                        _nop_ctr[0] += 1
                        n.engine = inst.engine
                        n.sync_info = bass_rust.SyncInfo(on_update=[], on_wait=[w])
                        insts.insert(i, n)
                        i += 1
                    inst.sync_info = bass_rust.SyncInfo(
                        on_update=list(si.on_update or []), on_wait=[keep]
                    )
                    n_fixed += 1
                i += 1
    return n_fixed


def _bcast_ap(dram_ap, reps):
    """Broadcast a DRAM AP over leading zero-step dims of sizes `reps`."""
    return AP(
        tensor=dram_ap.tensor,
        offset=dram_ap.offset,
        ap=[[0, r] for r in reps] + list(dram_ap.ap),
    )


# element offsets into the bf16 weight blob
OFF_WQ = 0
OFF_WK = 1 << 20
OFF_WV = 2 << 20
OFF_WO = 3 << 20
OFF_W1 = 4 << 20
OFF_W2 = 8 << 20
OFF_MISC = 12 << 20
OFF_BVR = OFF_MISC
OFF_B2R = OFF_MISC + 1024
OFF_G1 = OFF_MISC + 2048
OFF_BE1 = OFF_MISC + 3072
OFF_G2 = OFF_MISC + 4096
OFF_BE2 = OFF_MISC + 5120
OFF_ONEB = OFF_MISC + 6144
OFF_ONE1 = OFF_MISC + 6272
NWB = OFF_MISC + 8192


def _build():
    nc = bass.Bass()
    dp = nc.declare_dram_parameter

    # inputs are consolidated into 5 tensors: per-input staging overhead on
    # the device DMA rings (~7us x 2 per tensor) dominated the old 21-tensor
    # layout's first ~150us.
    WB = dp("wblob", [NWB], BF16, isOutput=False)
    SP = dp("srcp", [S, 1536], BF16, isOutput=False)       # [srcT | PT] per core
    SrcQ = dp("src_q", [SQ, D], F32, isOutput=False)       # natural query rows (+bo)
    FB = dp("fblob", [128, 48], F32, isOutput=False)       # bq2d|bk2d|b1_2d
    IDB = dp("identb", [129, 128], F32R, isOutput=False)   # eye(128); row128=ones64
    Out = dp("out", [SQ, D], F32, isOutput=True)

    def blob2d(off, r, c):
        return WB[off : off + r * c].rearrange("(r c) -> r c", c=c)

    Wq = blob2d(OFF_WQ, D, D)
    Wk = blob2d(OFF_WK, D, D)
    Wv = blob2d(OFF_WV, D, D)
    Wo = blob2d(OFF_WO, D, D)
    W1 = blob2d(OFF_W1, D, DFF)
    W2 = blob2d(OFF_W2, DFF, D)
    BvR = blob2d(OFF_BVR, 1, D)
    B2R = blob2d(OFF_B2R, 1, D)
    OnesB = blob2d(OFF_ONEB, 1, 128)

    with tile.TileContext(nc) as tc:
        import contextlib

        ctx = contextlib.ExitStack()
        with ctx:
            consts = ctx.enter_context(tc.tile_pool(name="consts", bufs=1))
            big1 = ctx.enter_context(tc.tile_pool(name="big1", bufs=1))
            big2 = ctx.enter_context(tc.tile_pool(name="big2", bufs=1))
            big3 = ctx.enter_context(tc.tile_pool(name="big3", bufs=1))
            ptp = ctx.enter_context(tc.tile_pool(name="ptp", bufs=1))
            vpool = ctx.enter_context(tc.tile_pool(name="vpool", bufs=1))
            resq = ctx.enter_context(tc.tile_pool(name="resq", bufs=1))
            xpool = ctx.enter_context(tc.tile_pool(name="xpool", bufs=1))
            wpool = ctx.enter_context(tc.tile_pool(name="wpool", bufs=2))
            w1pool = ctx.enter_context(tc.tile_pool(name="w1pool", bufs=3))
            w2pool = ctx.enter_context(tc.tile_pool(name="w2pool", bufs=2))
            epool = ctx.enter_context(tc.tile_pool(name="epool", bufs=3))
            lnpool = ctx.enter_context(tc.tile_pool(name="lnpool", bufs=1))
            small = ctx.enter_context(tc.tile_pool(name="small", bufs=2))
            ps = ctx.enter_context(tc.tile_pool(name="ps", bufs=1, space="PSUM"))

            # ---- constants ----
            ident = consts.tile([128, 128], F32R, tag="ident")
            nc.sync.dma_start(out=ident, in_=IDB[0:128, :])
            ones_b = consts.tile([1, 128], BF16, tag="ones_b")
            nc.sync.dma_start(out=ones_b, in_=OnesB[:, :])
            ones_f = consts.tile([1, 64], F32R, tag="ones_f")
            nc.sync.dma_start(out=ones_f, in_=IDB[128:129, 0:64])
            fall = consts.tile([128, 48], F32, tag="fall")
            nc.sync.dma_start(out=fall, in_=FB[:, :])
            bq2 = fall[:, 0:8]
            bk2 = fall[:, 8:16]
            b12 = fall[:, 16:48]
            bv_r = consts.tile([1, D], BF16, tag="bv_r")
            nc.sync.dma_start(out=bv_r, in_=BvR[:, :])
            b2_r = consts.tile([1, D], BF16, tag="b2_r")
            nc.sync.dma_start(out=b2_r, in_=B2R[:, :])
            g1b = consts.tile([128, D], BF16, tag="g1b")
            nc.sync.dma_start(out=g1b, in_=_bcast_ap(WB[OFF_G1 : OFF_G1 + D], [128]))
            be1b = consts.tile([128, D], BF16, tag="be1b")
            nc.sync.dma_start(out=be1b, in_=_bcast_ap(WB[OFF_BE1 : OFF_BE1 + D], [128]))
            ln_eps = consts.tile([128, 1], F32, tag="ln_eps")
            nc.vector.memset(ln_eps, EPS)

            # ---- srcT resident (keys on free dim, viewed [128, 2co+nt, 512]) ----
            st = big1.tile([128, 32, 512], BF16, tag="big1", name="st")
            for co in range(8):
                nc.sync.dma_start(
                    out=st[:, 2 * co : 2 * co + 2, :],
                    in_=SP[co * 128 : co * 128 + 128, 0:1024].rearrange(
                        "p (a m) -> p a m", a=2
                    ),
                )

            # small PE spin to cover initial DMA ramp
            spin = ps.tile([128, 512], F32, tag="gen", bufs=2, name="spin")
            for _ in range(8):
                nc.tensor.matmul(spin[:, 0:128], ident, ident, start=True, stop=True)

            def gen_psum(i, name="p"):
                return ps.tile(
                    [128, 512], F32, tag=("gen" if i % 2 == 0 else "pao"),
                    bufs=2, name=name,
                )

            # ---- K^T projection: kt[dout, keys] bf16 ----
            kt = big2.tile([128, 8, 1024], BF16, tag="big2", name="kt")
            pctr = 0
            for wc in range(2):
                wkbuf = wpool.tile([128, 8, 512], BF16, tag="w", name="wkbuf")
                nc.sync.dma_start(
                    out=wkbuf,
                    in_=Wk[:, wc * 512 : wc * 512 + 512].rearrange(
                        "(ko ki) m -> ki ko m", ki=128
                    ),
                )
                for dl in range(4):
                    dt = wc * 4 + dl
                    for nt in range(2):
                        psum = gen_psum(pctr); pctr += 1
                        for ko in range(8):
                            nc.tensor.matmul(
                                psum,
                                wkbuf[:, ko, dl * 128 : dl * 128 + 128],
                                st[:, 2 * ko + nt, :],
                                start=(ko == 0),
                                stop=(ko == 7),
                            )
                        nc.vector.tensor_scalar_add(
                            out=kt[:, dt, nt * 512 : nt * 512 + 512],
                            in0=psum,
                            scalar1=bk2[:, dt : dt + 1],
                        )

            # ---- Q^T projection: qt[dout, q] bf16 (Wq pre-scaled) ----
            qt = big3.tile([128, 8, 512], BF16, tag="big3", name="qt")
            for wc in range(2):
                wqbuf = wpool.tile([128, 8, 512], BF16, tag="w", name="wqbuf")
                nc.sync.dma_start(
                    out=wqbuf,
                    in_=Wq[:, wc * 512 : wc * 512 + 512].rearrange(
                        "(ko ki) m -> ki ko m", ki=128
                    ),
                )
                for dl in range(4):
                    dt = wc * 4 + dl
                    psum = gen_psum(pctr); pctr += 1
                    for ko in range(8):
                        nc.tensor.matmul(
                            psum,
                            wqbuf[:, ko, dl * 128 : dl * 128 + 128],
                            st[:, 2 * ko, :],
                            start=(ko == 0),
                            stop=(ko == 7),
                        )
                    nc.vector.tensor_scalar_add(
                        out=qt[:, dt, :], in0=psum, scalar1=bq2[:, dt : dt + 1]
                    )

            # ---- V projection (natural layout + ones column), bf16 ----
            v_sb = vpool.tile([128, 8, 16, 65], BF16, tag="v_sb")
            for mt in range(8):
                nc.gpsimd.dma_start(
                    out=v_sb[:, mt, :, 64:65],
                    in_=_bcast_ap(WB[OFF_ONE1 : OFF_ONE1 + 1], [128, 16]),
                )
            for vc in range(2):
                wvbuf = wpool.tile([128, 8, 512], BF16, tag="w", name="wvbuf")
                nc.sync.dma_start(
                    out=wvbuf,
                    in_=Wv[:, vc * 512 : vc * 512 + 512].rearrange(
                        "(ko ki) d -> ki ko d", ki=128
                    ),
                )
                for mt in range(8):
                    psum = gen_psum(pctr); pctr += 1
                    nc.tensor.matmul(
                        psum,
                        ones_b[0:1, 0:128],
                        bv_r[0:1, vc * 512 : vc * 512 + 512],
                        start=True,
                        stop=False,
                    )
                    for ko in range(8):
                        nc.tensor.matmul(
                            psum,
                            st[:, 2 * ko + mt // 4, (mt % 4) * 128 : (mt % 4) * 128 + 128],
                            wvbuf[:, ko, :],
                            start=False,
                            stop=(ko == 7),
                        )
                    nc.vector.tensor_copy(
                        out=v_sb[:, mt, vc * 8 : vc * 8 + 8, 0:64],
                        in_=psum.rearrange("p (h e) -> p h e", e=64),
                    )

            # ---- prefetch for post-attention phases (DMA-idle window) ----
            pt_sb = ptp.tile([128, 8, 512], BF16, tag="ptx", name="pt_sb")
            for ko in range(8):
                nc.sync.dma_start(
                    out=pt_sb[:, ko, :],
                    in_=SP[ko * 128 : ko * 128 + 128, 1024:1536],
                )
            g2b = consts.tile([128, D], BF16, tag="g2b")
            nc.sync.dma_start(out=g2b, in_=_bcast_ap(WB[OFF_G2 : OFF_G2 + D], [128]))
            be2b = consts.tile([128, D], BF16, tag="be2b")
            nc.sync.dma_start(out=be2b, in_=_bcast_ap(WB[OFF_BE2 : OFF_BE2 + D], [128]))
            src_q = resq.tile([128, 4, 1024], F32, tag="resq", name="src_q")
            for qt_i in range(4):
                nc.sync.dma_start(
                    out=src_q[:, qt_i, :],
                    in_=SrcQ[qt_i * 128 : qt_i * 128 + 128, :],
                )
            wobufs = []
            for nt in range(2):
                wobuf = wpool.tile([128, 8, 512], BF16, tag="w", name="wobuf")
                nc.sync.dma_start(
                    out=wobuf,
                    in_=Wo[:, nt * 512 : nt * 512 + 512].rearrange(
                        "(dp ki) d -> ki dp d", ki=128
                    ),
                )
                wobufs.append(wobuf)

            def load_w1(fc):
                w1buf = w1pool.tile([128, 8, 512], BF16, tag="w1", name="w1buf")
                nc.sync.dma_start(
                    out=w1buf,
                    in_=W1[:, fc * 512 : fc * 512 + 512].rearrange(
                        "(ko ki) f -> ki ko f", ki=128
                    ),
                )
                return w1buf

            def load_w2(c):
                w2c = w2pool.tile([128, 4, 1024], BF16, tag="w2", name="w2c")
                nc.sync.dma_start(
                    out=w2c,
                    in_=W2[c * 512 : c * 512 + 512, :].rearrange(
                        "(fo fi) d -> fi fo d", fi=128
                    ),
                )
                return w2c

            w1bufs = [load_w1(0), load_w1(1), load_w1(2)]
            w2cs = [load_w2(0), load_w2(1)]

            # ---- attention, head by head; ao written during attention into
            # the big1 slot (srcT is dead once V is done) ----
            ao_sb = big1.tile([128, 8, 512], BF16, tag="big1", name="ao_sb")
            for h in range(H):
                base = (h % 2) * 64
                dt = h // 2
                pao = ps.tile([128, 512], F32, tag="pao", bufs=2, name="pao")
                for kog in range(4):
                    pss = ps.tile([128, 2, 512], F32, tag="pss", bufs=2, name="pss")
                    for kl in range(2):
                        ko = kog * 2 + kl
                        nc.tensor.matmul(
                            pss[:, kl, :],
                            kt[base : base + 64, dt, ko * 128 : ko * 128 + 128],
                            qt[base : base + 64, dt, :],
                            start=True,
                            stop=True,
                        )
                    e_raw = epool.tile([128, 2, 512], BF16, tag="e_t", name="e_raw")
                    nc.scalar.activation(out=e_raw, in_=pss, func=FT.Exp)
                    e_t = epool.tile([128, 2, 512], BF16, tag="e_t", name="e_t")
                    nc.vector.tensor_mul(
                        out=e_t, in0=e_raw, in1=pt_sb[:, 2 * kog : 2 * kog + 2, :]
                    )
                    for kl in range(2):
                        ko = kog * 2 + kl
                        nc.tensor.matmul(
                            pao[0:65, :],
                            v_sb[:, ko, h, :],
                            e_t[:, kl, :],
                            start=(ko == 0),
                            stop=(ko == 7),
                        )
                # normalize: PE-broadcast the sums row to 64 partitions, then
                # a parallel 64-lane reciprocal (scalar Reciprocal is blocked)
                s_row = small.tile([1, 512], F32R, tag="s_row")
                nc.vector.tensor_copy(out=s_row, in_=pao[64:65, :])
                psb = ps.tile([128, 512], F32, tag="gen", bufs=2, name="psb")
                nc.tensor.matmul(
                    psb[0:64, :], ones_f[0:1, :], s_row, start=True, stop=True
                )
                rcpT = small.tile([64, 512], F32, tag="rcpT")
                nc.vector.reciprocal(out=rcpT, in_=psb[0:64, :])
                nc.vector.tensor_mul(
                    out=ao_sb[base : base + 64, dt, :], in0=pao[0:64, :], in1=rcpT
                )

            # ---- out projection + residual; x = LN1(src + ao@Wo + bo) ----
            x_sb = xpool.tile([128, 4, 1024], F32R, tag="x_sb")
            xT = ptp.tile([128, 8, 512], BF16, tag="ptx", name="xT")
            xpre_all = big2.tile([128, 4, 1024], F32, tag="big2", name="xpre_all")
            for nt in range(2):
                for qt_i in range(4):
                    psum = gen_psum(pctr); pctr += 1
                    for dpi in range(8):
                        nc.tensor.matmul(
                            psum,
                            ao_sb[:, dpi, qt_i * 128 : qt_i * 128 + 128],
                            wobufs[nt][:, dpi, :],
                            start=(dpi == 0),
                            stop=(dpi == 7),
                        )
                    nc.vector.tensor_add(
                        out=xpre_all[:, qt_i, nt * 512 : nt * 512 + 512],
                        in0=psum,
                        in1=src_q[:, qt_i, nt * 512 : nt * 512 + 512],
                    )
            for qt_i in range(4):
                xpre = xpre_all[:, qt_i, :]
                # LayerNorm 1
                stats = small.tile([128, 2, 6], F32, tag="stats")
                for half in range(2):
                    nc.vector.bn_stats(
                        out=stats[:, half, :],
                        in_=xpre[:, half * 512 : half * 512 + 512],
                    )
                mv = small.tile([128, 2], F32, tag="mv")
                nc.vector.bn_aggr(out=mv, in_=stats)
                sq = small.tile([128, 1], F32, tag="sq")
                nc.scalar.activation(
                    out=sq, in_=mv[:, 1:2], func=FT.Sqrt, bias=ln_eps
                )
                rstd = small.tile([128, 1], F32, tag="rstd")
                nc.vector.reciprocal(out=rstd, in_=sq)
                nmr = small.tile([128, 1], F32, tag="nmr")
                nc.vector.tensor_scalar(
                    out=nmr,
                    in0=mv[:, 0:1],
                    scalar1=rstd,
                    scalar2=-1.0,
                    op0=ALU.mult,
                    op1=ALU.mult,
                )
                xn = lnpool.tile([128, 1024], F32, tag="lnbig2")
                nc.scalar.activation(
                    out=xn, in_=xpre, func=FT.Identity, bias=nmr, scale=rstd
                )
                xg = lnpool.tile([128, 1024], F32, tag="lnbig")
                nc.gpsimd.tensor_mul(out=xg, in0=xn, in1=g1b)
                nc.vector.tensor_add(out=x_sb[:, qt_i, :], in0=xg, in1=be1b)
                for ct in range(8):
                    pt = ps.tile(
                        [128, 512], F32R,
                        tag=("gen" if ct % 2 == 0 else "pao"), bufs=2, name="pt",
                    )
                    nc.tensor.transpose(
                        pt[:, 0:128],
                        x_sb[:, qt_i, ct * 128 : ct * 128 + 128],
                        ident,
                    )
                    nc.vector.tensor_copy(
                        out=xT[:, ct, qt_i * 128 : qt_i * 128 + 128],
                        in_=pt[:, 0:128],
                    )

            # ---- FFN mm1 + relu: h[f, q] bf16 (big1 slot again) ----
            h_sb = big1.tile([128, 32, 512], BF16, tag="big1", name="h_sb")
            for fc in range(8):
                w1buf = w1bufs[fc] if fc < 3 else load_w1(fc)
                for fl in range(4):
                    ft = fc * 4 + fl
                    psum = gen_psum(pctr); pctr += 1
                    for ko in range(8):
                        nc.tensor.matmul(
                            psum,
                            w1buf[:, ko, fl * 128 : fl * 128 + 128],
                            xT[:, ko, :],
                            start=(ko == 0),
                            stop=(ko == 7),
                        )
                    nc.scalar.activation(
                        out=h_sb[:, ft, :],
                        in_=psum,
                        func=FT.Relu,
                        bias=b12[:, ft : ft + 1],
                    )

            # ---- FFN mm2 single-pass W2 stream; all 8 PSUM banks accumulate
            # (4 query tiles x 2 output halves); out = LN2(x + h@W2 + b2) ----
            pfA = ps.tile([128, 2, 512], F32, tag="pss", bufs=2, name="pfA")
            pfB = ps.tile([128, 2, 512], F32, tag="pss", bufs=2, name="pfB")
            pfC = ps.tile([128, 512], F32, tag="pao", bufs=2, name="pfC")
            pfD = ps.tile([128, 512], F32, tag="pao", bufs=2, name="pfD")
            pfE = ps.tile([128, 512], F32, tag="gen", bufs=2, name="pfE")
            pfF = ps.tile([128, 512], F32, tag="gen", bufs=2, name="pfF")
            psf = [
                pfA[:, 0, :], pfA[:, 1, :],
                pfB[:, 0, :], pfB[:, 1, :],
                pfC, pfD,
                pfE, pfF,
            ]
            for qt_i in range(4):
                for nt in range(2):
                    nc.tensor.matmul(
                        psf[qt_i * 2 + nt],
                        ones_b[0:1, 0:128],
                        b2_r[0:1, nt * 512 : nt * 512 + 512],
                        start=True,
                        stop=False,
                    )
            for c in range(8):
                w2c = w2cs[c] if c < 2 else load_w2(c)
                for j in range(4):
                    ft = c * 4 + j
                    for qt_i in range(4):
                        for nt in range(2):
                            nc.tensor.matmul(
                                psf[qt_i * 2 + nt],
                                h_sb[:, ft, qt_i * 128 : qt_i * 128 + 128],
                                w2c[:, j, nt * 512 : nt * 512 + 512],
                                start=False,
                                stop=(c == 7 and j == 3),
                            )

            ypre_all = resq.tile([128, 4, 1024], F32, tag="resq", name="ypre_all")
            for qt_i in range(4):
                for nt in range(2):
                    nc.vector.tensor_add(
                        out=ypre_all[:, qt_i, nt * 512 : nt * 512 + 512],
                        in0=psf[qt_i * 2 + nt],
                        in1=x_sb[:, qt_i, nt * 512 : nt * 512 + 512],
                    )
                ypre = ypre_all[:, qt_i, :]
                stats = small.tile([128, 2, 6], F32, tag="stats")
                for half in range(2):
                    nc.vector.bn_stats(
                        out=stats[:, half, :],
                        in_=ypre[:, half * 512 : half * 512 + 512],
                    )
                mv = small.tile([128, 2], F32, tag="mv")
                nc.vector.bn_aggr(out=mv, in_=stats)
                sq = small.tile([128, 1], F32, tag="sq")
                nc.scalar.activation(
                    out=sq, in_=mv[:, 1:2], func=FT.Sqrt, bias=ln_eps
                )
                rstd = small.tile([128, 1], F32, tag="rstd")
                nc.vector.reciprocal(out=rstd, in_=sq)
                nmr = small.tile([128, 1], F32, tag="nmr")
                nc.vector.tensor_scalar(
                    out=nmr,
                    in0=mv[:, 0:1],
                    scalar1=rstd,
                    scalar2=-1.0,
                    op0=ALU.mult,
                    op1=ALU.mult,
                )
                yn = lnpool.tile([128, 1024], F32, tag="lnbig")
                nc.scalar.activation(
                    out=yn, in_=ypre, func=FT.Identity, bias=nmr, scale=rstd
                )
                yg = lnpool.tile([128, 1024], F32, tag="lnbig2")
                nc.gpsimd.tensor_mul(out=yg, in0=yn, in1=g2b)
                out_t = lnpool.tile([128, 1024], F32, tag="lnbig")
                nc.vector.tensor_add(out=out_t, in0=yg, in1=be2b)
                nc.sync.dma_start(
                    out=Out[qt_i * 128 : qt_i * 128 + 128, :], in_=out_t
                )

    _legalize_waits(nc)
    return nc


_CACHE = {}


def kernel(**inputs):
    import os

    if "nc" not in _CACHE:
        _CACHE["nc"] = _build()
    nc = _CACHE["nc"]

    f32 = np.float32
    bf16 = ml_dtypes.bfloat16
    src = np.asarray(inputs["src"], f32)
    distances = np.asarray(inputs["distances"], f32)
    scale = np.float32(HD ** -0.5)
    bq_s = (np.asarray(inputs["bq"], f32) * scale).astype(f32)
    nabs = abs(float(np.asarray(inputs["dist_scale"])))

    wb = np.zeros(NWB, bf16)
    wb[OFF_WQ : OFF_WQ + D * D] = (np.asarray(inputs["Wq"], f32) * scale).astype(bf16).ravel()
    wb[OFF_WK : OFF_WK + D * D] = np.asarray(inputs["Wk"], f32).astype(bf16).ravel()
    wb[OFF_WV : OFF_WV + D * D] = np.asarray(inputs["Wv"], f32).astype(bf16).ravel()
    wb[OFF_WO : OFF_WO + D * D] = np.asarray(inputs["Wo"], f32).astype(bf16).ravel()
    wb[OFF_W1 : OFF_W1 + D * DFF] = np.asarray(inputs["W1"], f32).astype(bf16).ravel()
    wb[OFF_W2 : OFF_W2 + DFF * D] = np.asarray(inputs["W2"], f32).astype(bf16).ravel()
    wb[OFF_BVR : OFF_BVR + D] = np.asarray(inputs["bv"], f32).astype(bf16)
    wb[OFF_B2R : OFF_B2R + D] = np.asarray(inputs["b2"], f32).astype(bf16)
    wb[OFF_G1 : OFF_G1 + D] = np.asarray(inputs["g1"], f32).astype(bf16)
    wb[OFF_BE1 : OFF_BE1 + D] = np.asarray(inputs["beta1"], f32).astype(bf16)
    wb[OFF_G2 : OFF_G2 + D] = np.asarray(inputs["g2"], f32).astype(bf16)
    wb[OFF_BE2 : OFF_BE2 + D] = np.asarray(inputs["beta2"], f32).astype(bf16)
    wb[OFF_ONEB : OFF_ONEB + 128] = np.ones(128, bf16)
    wb[OFF_ONE1] = bf16(1.0)

    fblob = np.concatenate(
        [
            np.ascontiguousarray(bq_s.reshape(8, 128).T),
            np.ascontiguousarray(np.asarray(inputs["bk"], f32).reshape(8, 128).T),
            np.ascontiguousarray(np.asarray(inputs["b1"], f32).reshape(32, 128).T),
        ],
        axis=1,
    ).astype(f32)
    identb = np.concatenate(
        [
            np.eye(128, dtype=f32),
            np.concatenate([np.ones((1, 64), f32), np.zeros((1, 64), f32)], axis=1),
        ],
        axis=0,
    )

    shared = {"wblob": wb, "fblob": fblob, "identb": identb}

    in_maps = []
    for c in range(NCORES):
        b, qh = c // 2, c % 2
        q0 = qh * SQ
        if qh == 0:
            perm = np.arange(S)
        else:
            perm = np.r_[np.arange(512, 1024), np.arange(0, 512)]
        m = dict(shared)
        srcT = np.ascontiguousarray(src[b][perm].T).astype(bf16)
        dT = np.ascontiguousarray(distances[b, q0 : q0 + SQ][:, perm].T)
        ptb = np.exp(np.log(dT + np.float32(1e-9)) * np.float32(-nabs)).astype(bf16)
        m["srcp"] = np.ascontiguousarray(np.concatenate([srcT, ptb], axis=1))
        m["src_q"] = np.ascontiguousarray(
            src[b, q0 : q0 + SQ] + np.asarray(inputs["bo"], f32)[None, :]
        )
        in_maps.append(m)

    trace = bool(int(os.environ.get("BASS_KERNEL_TRACE", "0")))
    res = run_bass_kernel_spmd(
        nc,
        in_maps,
        core_ids=list(range(NCORES)),
        trace=trace,
        stitch_traces=False,
    )
    _CACHE["last_result"] = res

    out = np.empty((B, S, D), f32)
    for c in range(NCORES):
        b, qh = c // 2, c % 2
        out[b, qh * SQ : qh * SQ + SQ] = res.results[c]["out"]
    return out


# revision 24
# speedup vs baseline: 1.0452x; 1.0452x over previous
"""Distance-aware transformer encoder layer on 8 Trainium2 NeuronCores.

Sharding: core c handles batch b = c//2 and query-half qh = c%2 (512 query
rows). K/V are computed per-core for the full 1024-key sequence of its batch
(duplicated across the core pair — cheaper than a collective). Everything
else (scores, softmax, out-proj, LayerNorms, FFN) is perfectly sharded by
query rows. No collectives.

Precision: bf16 operands for every matmul (fp32 PSUM accumulation); fp32
residual path (src_q, xpre, x, ypre) and LayerNorm statistics. The distance
bias is folded in exp-space: host ships P^T = (d+1e-9)^(-|dist_scale|) as
bf16 and the kernel computes E = exp(scores) * P on Scalar+Vector, which
removes the per-head identity-matmul bias injection of the earlier version.
Softmax normalization broadcasts 1/rowsum via a 1-row PE matmul instead of
a DRAM round-trip. W2 streams through SBUF once (8 PSUM banks accumulate
all 4 query tiles x 2 output halves simultaneously).
"""

import numpy as np
import ml_dtypes

import bass_rust
import concourse.bass as bass
import concourse.tile as tile
import concourse.mybir as mybir
from concourse.bass import AP
from concourse.bass_utils import run_bass_kernel_spmd

B, S, D, H, DFF, HD = 4, 1024, 1024, 16, 4096, 64
SQ = 512          # query rows per core
NCORES = 8
EPS = 1e-5
F32 = mybir.dt.float32
F32R = mybir.dt.float32r
BF16 = mybir.dt.bfloat16
FT = mybir.ActivationFunctionType
ALU = mybir.AluOpType

_nop_ctr = [0]


def _legalize_waits(nc):
    """walrus codegen in this toolchain accepts only one sync-wait per
    instruction; split extras onto same-engine NoOps inserted before."""
    n_fixed = 0
    for f in nc.m.functions:
        for bb in f.blocks:
            insts = bb.instructions
            i = 0
            while i < len(insts):
                inst = insts[i]
                si = inst.sync_info
                waits = list(si.on_wait) if si is not None and si.on_wait else []
                if len(waits) > 1:
                    keep = waits[-1]
                    for w in waits[:-1]:
                        n = bass_rust.InstNoOp(
                            name=f"waitsplit-nop-{_nop_ctr[0]}", ins=[], outs=[]
                        )
                        _nop_ctr[0] += 1
                        n.engine = inst.engine
                        n.sync_info = bass_rust.SyncInfo(on_update=[], on_wait=[w])
                        insts.insert(i, n)
                        i += 1
                    inst.sync_info = bass_rust.SyncInfo(
                        on_update=list(si.on_update or []), on_wait=[keep]
                    )
                    n_fixed += 1
                i += 1
    return n_fixed


def _bcast_ap(dram_ap, reps):
    """Broadcast a DRAM AP over leading zero-step dims of sizes `reps`."""
    return AP(
        tensor=dram_ap.tensor,
        offset=dram_ap.offset,
        ap=[[0, r] for r in reps] + list(dram_ap.ap),
    )


# element offsets into the bf16 weight blob
OFF_WQ = 0
OFF_WK = 1 << 20
OFF_WV = 2 << 20
OFF_WO = 3 << 20
OFF_W1 = 4 << 20
OFF_W2 = 8 << 20
OFF_MISC = 12 << 20
OFF_BVR = OFF_MISC
OFF_B2R = OFF_MISC + 1024
OFF_G1 = OFF_MISC + 2048
OFF_BE1 = OFF_MISC + 3072
OFF_G2 = OFF_MISC + 4096
OFF_BE2 = OFF_MISC + 5120
OFF_ONEB = OFF_MISC + 6144
OFF_ONE1 = OFF_MISC + 6272
NWB = OFF_MISC + 8192


def _build():
    nc = bass.Bass()
    dp = nc.declare_dram_parameter

    # inputs are consolidated into 5 tensors: per-input staging overhead on
    # the device DMA rings (~7us x 2 per tensor) dominated the old 21-tensor
    # layout's first ~150us.
    WB = dp("wblob", [NWB], BF16, isOutput=False)
    SP = dp("srcp", [S, 1536], BF16, isOutput=False)       # [srcT | PT] per core
    SrcQ = dp("src_q", [SQ, D], F32, isOutput=False)       # natural query rows (+bo)
    FB = dp("fblob", [128, 48], F32, isOutput=False)       # bq2d|bk2d|b1_2d
    IDB = dp("identb", [129, 128], F32R, isOutput=False)   # eye(128); row128=ones64
    Out = dp("out", [SQ, D], F32, isOutput=True)

    def blob2d(off, r, c):
        return WB[off : off + r * c].rearrange("(r c) -> r c", c=c)

    Wq = blob2d(OFF_WQ, D, D)
    Wk = blob2d(OFF_WK, D, D)
    Wv = blob2d(OFF_WV, D, D)
    Wo = blob2d(OFF_WO, D, D)
    W1 = blob2d(OFF_W1, D, DFF)
    W2 = blob2d(OFF_W2, DFF, D)
    BvR = blob2d(OFF_BVR, 1, D)
    B2R = blob2d(OFF_B2R, 1, D)
    OnesB = blob2d(OFF_ONEB, 1, 128)

    with tile.TileContext(nc) as tc:
        import contextlib

        ctx = contextlib.ExitStack()
        with ctx:
            consts = ctx.enter_context(tc.tile_pool(name="consts", bufs=1))
            big1 = ctx.enter_context(tc.tile_pool(name="big1", bufs=1))
            big2 = ctx.enter_context(tc.tile_pool(name="big2", bufs=1))
            big3 = ctx.enter_context(tc.tile_pool(name="big3", bufs=1))
            ptp = ctx.enter_context(tc.tile_pool(name="ptp", bufs=1))
            vpool = ctx.enter_context(tc.tile_pool(name="vpool", bufs=1))
            resq = ctx.enter_context(tc.tile_pool(name="resq", bufs=1))
            xpool = ctx.enter_context(tc.tile_pool(name="xpool", bufs=1))
            wpool = ctx.enter_context(tc.tile_pool(name="wpool", bufs=2))
            w1pool = ctx.enter_context(tc.tile_pool(name="w1pool", bufs=3))
            w2pool = ctx.enter_context(tc.tile_pool(name="w2pool", bufs=2))
            epool = ctx.enter_context(tc.tile_pool(name="epool", bufs=3))
            lnpool = ctx.enter_context(tc.tile_pool(name="lnpool", bufs=1))
            small = ctx.enter_context(tc.tile_pool(name="small", bufs=2))
            dscratch = ctx.enter_context(tc.tile_pool(name="dscratch", bufs=4, space="DRAM"))
            ps = ctx.enter_context(tc.tile_pool(name="ps", bufs=1, space="PSUM"))

            # ---- constants ----
            ident = consts.tile([128, 128], F32R, tag="ident")
            nc.sync.dma_start(out=ident, in_=IDB[0:128, :])
            ones_b = consts.tile([1, 128], BF16, tag="ones_b")
            nc.sync.dma_start(out=ones_b, in_=OnesB[:, :])
            ones_f = consts.tile([1, 64], F32R, tag="ones_f")
            nc.sync.dma_start(out=ones_f, in_=IDB[128:129, 0:64])
            fall = consts.tile([128, 48], F32, tag="fall")
            nc.sync.dma_start(out=fall, in_=FB[:, :])
            bq2 = fall[:, 0:8]
            bk2 = fall[:, 8:16]
            b12 = fall[:, 16:48]
            bv_r = consts.tile([1, D], BF16, tag="bv_r")
            nc.sync.dma_start(out=bv_r, in_=BvR[:, :])
            b2_r = consts.tile([1, D], BF16, tag="b2_r")
            nc.sync.dma_start(out=b2_r, in_=B2R[:, :])
            g1b = consts.tile([128, D], BF16, tag="g1b")
            nc.sync.dma_start(out=g1b, in_=_bcast_ap(WB[OFF_G1 : OFF_G1 + D], [128]))
            be1b = consts.tile([128, D], BF16, tag="be1b")
            nc.sync.dma_start(out=be1b, in_=_bcast_ap(WB[OFF_BE1 : OFF_BE1 + D], [128]))
            ln_eps = consts.tile([128, 1], F32, tag="ln_eps")
            nc.vector.memset(ln_eps, EPS)

            # ---- srcT resident (keys on free dim, viewed [128, 2co+nt, 512]) ----
            st = big1.tile([128, 32, 512], BF16, tag="big1", name="st")
            for co in range(8):
                nc.sync.dma_start(
                    out=st[:, 2 * co : 2 * co + 2, :],
                    in_=SP[co * 128 : co * 128 + 128, 0:1024].rearrange(
                        "p (a m) -> p a m", a=2
                    ),
                )

            # small PE spin to cover initial DMA ramp
            spin = ps.tile([128, 512], F32, tag="pao", bufs=2, name="spin")
            for _ in range(8):
                nc.tensor.matmul(spin[:, 0:128], ident, ident, start=True, stop=True)

            # PSUM: "pss" [128,2,512] x3 (6 banks) + "pao" [128,512] x2 = 8.
            # General-purpose psums hand out halves of pss tiles.
            _gp = {"t": None, "i": 0}

            def gen_psum(i=None, name="p"):
                if _gp["i"] % 2 == 0:
                    _gp["t"] = ps.tile(
                        [128, 2, 512], F32, tag="pss", bufs=3, name=name
                    )
                t = _gp["t"][:, _gp["i"] % 2, :]
                _gp["i"] += 1
                return t

            # ---- K^T projection: kt[dout, keys] bf16 ----
            kt = big2.tile([128, 8, 1024], BF16, tag="big2", name="kt")
            for wc in range(2):
                wkbuf = wpool.tile([128, 8, 512], BF16, tag="w", name="wkbuf")
                nc.sync.dma_start(
                    out=wkbuf,
                    in_=Wk[:, wc * 512 : wc * 512 + 512].rearrange(
                        "(ko ki) m -> ki ko m", ki=128
                    ),
                )
                for dl in range(4):
                    dt = wc * 4 + dl
                    for nt in range(2):
                        psum = gen_psum()
                        for ko in range(8):
                            nc.tensor.matmul(
                                psum,
                                wkbuf[:, ko, dl * 128 : dl * 128 + 128],
                                st[:, 2 * ko + nt, :],
                                start=(ko == 0),
                                stop=(ko == 7),
                            )
                        nc.vector.tensor_scalar_add(
                            out=kt[:, dt, nt * 512 : nt * 512 + 512],
                            in0=psum,
                            scalar1=bk2[:, dt : dt + 1],
                        )

            # ---- Q^T projection: qt[dout, q] bf16 (Wq pre-scaled) ----
            qt = big3.tile([128, 8, 512], BF16, tag="big3", name="qt")
            for wc in range(2):
                wqbuf = wpool.tile([128, 8, 512], BF16, tag="w", name="wqbuf")
                nc.sync.dma_start(
                    out=wqbuf,
                    in_=Wq[:, wc * 512 : wc * 512 + 512].rearrange(
                        "(ko ki) m -> ki ko m", ki=128
                    ),
                )
                for dl in range(4):
                    dt = wc * 4 + dl
                    psum = gen_psum()
                    for ko in range(8):
                        nc.tensor.matmul(
                            psum,
                            wqbuf[:, ko, dl * 128 : dl * 128 + 128],
                            st[:, 2 * ko, :],
                            start=(ko == 0),
                            stop=(ko == 7),
                        )
                    nc.vector.tensor_scalar_add(
                        out=qt[:, dt, :], in0=psum, scalar1=bq2[:, dt : dt + 1]
                    )

            # ---- V projection (natural layout + ones column), bf16 ----
            v_sb = vpool.tile([128, 8, 16, 65], BF16, tag="v_sb")
            for mt in range(8):
                nc.gpsimd.dma_start(
                    out=v_sb[:, mt, :, 64:65],
                    in_=_bcast_ap(WB[OFF_ONE1 : OFF_ONE1 + 1], [128, 16]),
                )
            def load_wv(vc):
                wvbuf = wpool.tile([128, 8, 512], BF16, tag="w", name="wvbuf")
                nc.sync.dma_start(
                    out=wvbuf,
                    in_=Wv[:, vc * 512 : vc * 512 + 512].rearrange(
                        "(ko ki) d -> ki ko d", ki=128
                    ),
                )
                return wvbuf

            def v_proj_mt(vc, wvbuf, mt, own_tile=False):
                if own_tile:
                    # whole pss tile per call: a half-tile shared across two
                    # heads deadlocks against the score-tile rotation
                    psum = ps.tile(
                        [128, 2, 512], F32, tag="pss", bufs=3, name="vp"
                    )[:, 0, :]
                else:
                    psum = gen_psum()
                nc.tensor.matmul(
                    psum,
                    ones_b[0:1, 0:128],
                    bv_r[0:1, vc * 512 : vc * 512 + 512],
                    start=True,
                    stop=False,
                )
                for ko in range(8):
                    nc.tensor.matmul(
                        psum,
                        st[:, 2 * ko + mt // 4, (mt % 4) * 128 : (mt % 4) * 128 + 128],
                        wvbuf[:, ko, :],
                        start=False,
                        stop=(ko == 7),
                    )
                nc.vector.tensor_copy(
                    out=v_sb[:, mt, vc * 8 : vc * 8 + 8, 0:64],
                    in_=psum.rearrange("p (h e) -> p h e", e=64),
                )

            wvbuf0 = load_wv(0)
            for mt in range(8):
                v_proj_mt(0, wvbuf0, mt)
            wvbuf1 = load_wv(1)
            for mt in range(8):
                v_proj_mt(1, wvbuf1, mt)

            # ---- prefetch for post-attention phases (DMA-idle window) ----
            pt_sb = ptp.tile([128, 8, 512], BF16, tag="ptx", name="pt_sb")
            for ko in range(8):
                nc.sync.dma_start(
                    out=pt_sb[:, ko, :],
                    in_=SP[ko * 128 : ko * 128 + 128, 1024:1536],
                )
            src_q = resq.tile([128, 4, 1024], F32, tag="resq", name="src_q")
            for qt_i in range(4):
                nc.sync.dma_start(
                    out=src_q[:, qt_i, :],
                    in_=SrcQ[qt_i * 128 : qt_i * 128 + 128, :],
                )
            wobufs = []
            for nt in range(2):
                wobuf = wpool.tile([128, 8, 512], BF16, tag="w", name="wobuf")
                nc.sync.dma_start(
                    out=wobuf,
                    in_=Wo[:, nt * 512 : nt * 512 + 512].rearrange(
                        "(dp ki) d -> ki dp d", ki=128
                    ),
                )
                wobufs.append(wobuf)

            def load_w1(fc):
                w1buf = w1pool.tile([128, 8, 512], BF16, tag="w1", name="w1buf")
                nc.sync.dma_start(
                    out=w1buf,
                    in_=W1[:, fc * 512 : fc * 512 + 512].rearrange(
                        "(ko ki) f -> ki ko f", ki=128
                    ),
                )
                return w1buf

            def load_w2(c):
                w2c = w2pool.tile([128, 4, 1024], BF16, tag="w2", name="w2c")
                nc.sync.dma_start(
                    out=w2c,
                    in_=W2[c * 512 : c * 512 + 512, :].rearrange(
                        "(fo fi) d -> fi fo d", fi=128
                    ),
                )
                return w2c

            w1bufs = [load_w1(0), load_w1(1), load_w1(2)]
            w2cs = [load_w2(0), load_w2(1)]

            # ---- attention, head by head; ao written during attention into
            # the big1 slot (srcT is dead once V is done). V chunk 1 (heads
            # 8-15) matmuls interleave into heads 0-7 to keep the PE hot
            # through the exp/mul latency; heads 8-15 get ident-spin filler.
            # Normalization: sums row PSUM->DRAM->bcast, then one GpSimd
            # divide per head (Scalar stays exp-only, Vector stays mul-only).
            ao_sb = big1.tile([128, 8, 512], BF16, tag="big1", name="ao_sb")
            # sums rows collect at partition bases {0,64} (legal single-
            # partition bases), 4 heads per batch; each batch flushes a
            # reciprocal to DRAM. drcp layout: [h%2, dpi, q].
            s_all = small.tile([128, 2, 512], F32, tag="s_all", bufs=1)
            drcp = dscratch.tile([2, 8, 512], BF16, tag="drcp")
            for h in range(H):
                base = (h % 2) * 64
                dt = h // 2
                pao = ps.tile([128, 512], F32, tag="pao", bufs=2, name="pao")
                psss = []
                for kog in range(4):
                    pss = ps.tile([128, 2, 512], F32, tag="pss", bufs=3, name="pss")
                    psss.append(pss)
                    for kl in range(2):
                        ko = kog * 2 + kl
                        nc.tensor.matmul(
                            pss[:, kl, :],
                            kt[base : base + 64, dt, ko * 128 : ko * 128 + 128],
                            qt[base : base + 64, dt, :],
                            start=True,
                            stop=True,
                        )
                    e_raw = epool.tile([128, 2, 512], BF16, tag="e_t", name="e_raw")
                    nc.scalar.activation(out=e_raw, in_=pss, func=FT.Exp)
                    e_t = epool.tile([128, 2, 512], BF16, tag="e_t", name="e_t")
                    nc.vector.tensor_mul(
                        out=e_t, in0=e_raw, in1=pt_sb[:, 2 * kog : 2 * kog + 2, :]
                    )
                    for kl in range(2):
                        ko = kog * 2 + kl
                        nc.tensor.matmul(
                            pao[0:65, :],
                            v_sb[:, ko, h, :],
                            e_t[:, kl, :],
                            start=(ko == 0),
                            stop=(ko == 7),
                        )
                nc.vector.tensor_copy(
                    out=ao_sb[base : base + 64, dt, :], in_=pao[0:64, :]
                )
                b0 = 64 * (h % 2)
                nc.vector.tensor_copy(
                    out=s_all[b0 : b0 + 1, (h // 2) % 2, :], in_=pao[64:65, :]
                )
                if h % 4 == 3:
                    k = h // 4
                    rcp_bf = small.tile([128, 2, 512], BF16, tag="rcp_bf", bufs=2)
                    with nc.allow_low_precision(reason="softmax denom in bf16, ~0.4% scale error within tolerance"):
                        nc.vector.reciprocal(out=rcp_bf, in_=s_all)
                    for h1 in range(2):
                        nc.sync.dma_start(
                            out=drcp[h1, 2 * k : 2 * k + 2, :],
                            in_=rcp_bf[64 * h1 : 64 * h1 + 1, :, :],
                        )

            # softmax denominators were reciprocal'd in two halves above;
            # broadcast them to [128(dims) x head-pair] and normalize in one op
            rcpb = ptp.tile([128, 8, 512], BF16, tag="ptx", name="rcpb")
            nc.sync.dma_start(
                out=rcpb[0:64, :, :], in_=_bcast_ap(drcp[0, :, :], [64])
            )
            nc.sync.dma_start(
                out=rcpb[64:128, :, :], in_=_bcast_ap(drcp[1, :, :], [64])
            )
            spin2 = ps.tile([128, 512], F32, tag="pao", bufs=2, name="spin2")
            for _ in range(20):
                nc.tensor.matmul(spin2[:, 0:128], ident, ident, start=True, stop=True)
            ao_n = big3.tile([128, 8, 512], BF16, tag="big3", name="ao_n")
            nc.vector.tensor_mul(out=ao_n, in0=ao_sb, in1=rcpb)

            # ---- out projection + residual; x = LN1(src + ao@Wo + bo).
            # g1/beta1 are folded into W1/b1 on the host, so FFN1 consumes the
            # pre-affine normalized xn directly; x_sb (residual for FFN2) gets
            # the affine applied off the critical path on GpSimd/Vector. ----
            x_sb = xpool.tile([128, 4, 1024], F32R, tag="x_sb")
            xT = ptp.tile([128, 8, 512], BF16, tag="ptx", name="xT")
            xpre_all = big2.tile([128, 4, 1024], F32, tag="big2", name="xpre_all")
            xns = []
            for qt_i in range(4):
                for nt in range(2):
                    psum = gen_psum()
                    for dpi in range(8):
                        nc.tensor.matmul(
                            psum,
                            ao_n[:, dpi, qt_i * 128 : qt_i * 128 + 128],
                            wobufs[nt][:, dpi, :],
                            start=(dpi == 0),
                            stop=(dpi == 7),
                        )
                    nc.vector.tensor_add(
                        out=xpre_all[:, qt_i, nt * 512 : nt * 512 + 512],
                        in0=psum,
                        in1=src_q[:, qt_i, nt * 512 : nt * 512 + 512],
                    )
                xpre = xpre_all[:, qt_i, :]
                stats = small.tile([128, 2, 6], F32, tag="stats", bufs=4)
                for half in range(2):
                    nc.vector.bn_stats(
                        out=stats[:, half, :],
                        in_=xpre[:, half * 512 : half * 512 + 512],
                    )
                mv = small.tile([128, 2], F32, tag="mv", bufs=4)
                nc.vector.bn_aggr(out=mv, in_=stats)
                sq = small.tile([128, 1], F32, tag="sq", bufs=4)
                nc.scalar.activation(
                    out=sq, in_=mv[:, 1:2], func=FT.Sqrt, bias=ln_eps
                )
                rstd = small.tile([128, 1], F32, tag="rstd", bufs=4)
                nc.vector.reciprocal(out=rstd, in_=sq)
                nmr = small.tile([128, 1], F32, tag="nmr", bufs=4)
                nc.vector.tensor_scalar(
                    out=nmr,
                    in0=mv[:, 0:1],
                    scalar1=rstd,
                    scalar2=-1.0,
                    op0=ALU.mult,
                    op1=ALU.mult,
                )
                xn = lnpool.tile([128, 1024], F32R, tag="xn", bufs=2, name="xn")
                nc.scalar.activation(
                    out=xn, in_=xpre, func=FT.Identity, bias=nmr, scale=rstd
                )
                xns.append(xn)
            for qt_i in range(4):
                for ct in range(8):
                    pt = ps.tile([128, 512], F32R, tag="pao", bufs=2, name="pt")
                    nc.tensor.transpose(
                        pt[:, 0:128],
                        xns[qt_i][:, ct * 128 : ct * 128 + 128],
                        ident,
                    )
                    nc.vector.tensor_copy(
                        out=xT[:, ct, qt_i * 128 : qt_i * 128 + 128],
                        in_=pt[:, 0:128],
                    )
                xg = lnpool.tile([128, 1024], F32, tag="lnbig", bufs=1, name="xg")
                nc.gpsimd.tensor_mul(out=xg, in0=xns[qt_i], in1=g1b)
                nc.vector.tensor_add(out=x_sb[:, qt_i, :], in0=xg, in1=be1b)

            g2b = consts.tile([128, D], BF16, tag="g1b", name="g2b")
            nc.sync.dma_start(out=g2b, in_=_bcast_ap(WB[OFF_G2 : OFF_G2 + D], [128]))
            be2b = consts.tile([128, D], BF16, tag="be1b", name="be2b")
            nc.sync.dma_start(out=be2b, in_=_bcast_ap(WB[OFF_BE2 : OFF_BE2 + D], [128]))

            # ---- FFN mm1 + relu: h[f, q] bf16 (big1 slot again) ----
            h_sb = big1.tile([128, 32, 512], BF16, tag="big1", name="h_sb")
            for fc in range(8):
                w1buf = w1bufs[fc] if fc < 3 else load_w1(fc)
                for fl in range(4):
                    ft = fc * 4 + fl
                    psum = gen_psum()
                    for ko in range(8):
                        nc.tensor.matmul(
                            psum,
                            w1buf[:, ko, fl * 128 : fl * 128 + 128],
                            xT[:, ko, :],
                            start=(ko == 0),
                            stop=(ko == 7),
                        )
                    nc.scalar.activation(
                        out=h_sb[:, ft, :],
                        in_=psum,
                        func=FT.Relu,
                        bias=b12[:, ft : ft + 1],
                    )

            # ---- FFN mm2 single-pass W2 stream; all 8 PSUM banks accumulate
            # (4 query tiles x 2 output halves); out = LN2(x + h@W2 + b2) ----
            pfA = ps.tile([128, 2, 512], F32, tag="pss", bufs=3, name="pfA")
            pfB = ps.tile([128, 2, 512], F32, tag="pss", bufs=3, name="pfB")
            pfC = ps.tile([128, 2, 512], F32, tag="pss", bufs=3, name="pfC")
            pfD = ps.tile([128, 512], F32, tag="pao", bufs=2, name="pfD")
            pfE = ps.tile([128, 512], F32, tag="pao", bufs=2, name="pfE")
            psf = [
                pfA[:, 0, :], pfA[:, 1, :],
                pfB[:, 0, :], pfB[:, 1, :],
                pfC[:, 0, :], pfC[:, 1, :],
                pfD, pfE,
            ]
            for qt_i in range(4):
                for nt in range(2):
                    nc.tensor.matmul(
                        psf[qt_i * 2 + nt],
                        ones_b[0:1, 0:128],
                        b2_r[0:1, nt * 512 : nt * 512 + 512],
                        start=True,
                        stop=False,
                    )
            for c in range(8):
                w2c = w2cs[c] if c < 2 else load_w2(c)
                if c < 7:
                    for j in range(4):
                        ft = c * 4 + j
                        for qt_i in range(4):
                            for nt in range(2):
                                nc.tensor.matmul(
                                    psf[qt_i * 2 + nt],
                                    h_sb[:, ft, qt_i * 128 : qt_i * 128 + 128],
                                    w2c[:, j, nt * 512 : nt * 512 + 512],
                                    start=False,
                                    stop=False,
                                )
                else:
                    # last chunk: qt-major so early query tiles stop first and
                    # their LayerNorm/output drains under the remaining matmuls
                    for qt_i in range(4):
                        for j in range(4):
                            ft = c * 4 + j
                            for nt in range(2):
                                nc.tensor.matmul(
                                    psf[qt_i * 2 + nt],
                                    h_sb[:, ft, qt_i * 128 : qt_i * 128 + 128],
                                    w2c[:, j, nt * 512 : nt * 512 + 512],
                                    start=False,
                                    stop=(j == 3),
                                )

            ypre_all = resq.tile([128, 4, 1024], F32, tag="resq", name="ypre_all")
            yns = []
            for qt_i in range(4):
                for nt in range(2):
                    nc.vector.tensor_add(
                        out=ypre_all[:, qt_i, nt * 512 : nt * 512 + 512],
                        in0=psf[qt_i * 2 + nt],
                        in1=x_sb[:, qt_i, nt * 512 : nt * 512 + 512],
                    )
                ypre = ypre_all[:, qt_i, :]
                stats = small.tile([128, 2, 6], F32, tag="stats", bufs=4)
                for half in range(2):
                    nc.vector.bn_stats(
                        out=stats[:, half, :],
                        in_=ypre[:, half * 512 : half * 512 + 512],
                    )
                mv = small.tile([128, 2], F32, tag="mv", bufs=4)
                nc.vector.bn_aggr(out=mv, in_=stats)
                sq = small.tile([128, 1], F32, tag="sq", bufs=4)
                nc.scalar.activation(
                    out=sq, in_=mv[:, 1:2], func=FT.Sqrt, bias=ln_eps
                )
                rstd = small.tile([128, 1], F32, tag="rstd", bufs=4)
                nc.vector.reciprocal(out=rstd, in_=sq)
                nmr = small.tile([128, 1], F32, tag="nmr", bufs=4)
                nc.vector.tensor_scalar(
                    out=nmr,
                    in0=mv[:, 0:1],
                    scalar1=rstd,
                    scalar2=-1.0,
                    op0=ALU.mult,
                    op1=ALU.mult,
                )
                yn = lnpool.tile([128, 1024], F32, tag="xn", bufs=2, name="yn")
                nc.scalar.activation(
                    out=yn, in_=ypre, func=FT.Identity, bias=nmr, scale=rstd
                )
                yns.append(yn)
            for qt_i in range(4):
                yg = lnpool.tile([128, 1024], F32, tag="lnbig", bufs=1, name="yg")
                nc.gpsimd.tensor_mul(out=yg, in0=yns[qt_i], in1=g2b)
                # final add lands in the (dead) ypre region; no extra slot
                nc.vector.tensor_add(
                    out=ypre_all[:, qt_i, :], in0=yg, in1=be2b
                )
                nc.sync.dma_start(
                    out=Out[qt_i * 128 : qt_i * 128 + 128, :],
                    in_=ypre_all[:, qt_i, :],
                )

    _legalize_waits(nc)
    return nc


_CACHE = {}


def kernel(**inputs):
    import os

    if "nc" not in _CACHE:
        _CACHE["nc"] = _build()
    nc = _CACHE["nc"]

    f32 = np.float32
    bf16 = ml_dtypes.bfloat16
    src = np.asarray(inputs["src"], f32)
    distances = np.asarray(inputs["distances"], f32)
    scale = np.float32(HD ** -0.5)
    bq_s = (np.asarray(inputs["bq"], f32) * scale).astype(f32)
    nabs = abs(float(np.asarray(inputs["dist_scale"])))

    wb = np.zeros(NWB, bf16)
    wb[OFF_WQ : OFF_WQ + D * D] = (np.asarray(inputs["Wq"], f32) * scale).astype(bf16).ravel()
    wb[OFF_WK : OFF_WK + D * D] = np.asarray(inputs["Wk"], f32).astype(bf16).ravel()
    wb[OFF_WV : OFF_WV + D * D] = np.asarray(inputs["Wv"], f32).astype(bf16).ravel()
    wb[OFF_WO : OFF_WO + D * D] = np.asarray(inputs["Wo"], f32).astype(bf16).ravel()
    W1f = np.asarray(inputs["W1"], f32)
    g1f = np.asarray(inputs["g1"], f32)
    be1f = np.asarray(inputs["beta1"], f32)
    # LN1's affine is folded into the FFN first layer: relu(x@W1+b1) with
    # x = xn*g1+be1  ==  relu(xn@(g1[:,None]*W1) + (b1 + be1@W1))
    wb[OFF_W1 : OFF_W1 + D * DFF] = (W1f * g1f[:, None]).astype(bf16).ravel()
    wb[OFF_W2 : OFF_W2 + DFF * D] = np.asarray(inputs["W2"], f32).astype(bf16).ravel()
    wb[OFF_BVR : OFF_BVR + D] = np.asarray(inputs["bv"], f32).astype(bf16)
    wb[OFF_B2R : OFF_B2R + D] = np.asarray(inputs["b2"], f32).astype(bf16)
    wb[OFF_G1 : OFF_G1 + D] = np.asarray(inputs["g1"], f32).astype(bf16)
    wb[OFF_BE1 : OFF_BE1 + D] = np.asarray(inputs["beta1"], f32).astype(bf16)
    wb[OFF_G2 : OFF_G2 + D] = np.asarray(inputs["g2"], f32).astype(bf16)
    wb[OFF_BE2 : OFF_BE2 + D] = np.asarray(inputs["beta2"], f32).astype(bf16)
    wb[OFF_ONEB : OFF_ONEB + 128] = np.ones(128, bf16)
    wb[OFF_ONE1] = bf16(1.0)

    fblob = np.concatenate(
        [
            np.ascontiguousarray(bq_s.reshape(8, 128).T),
            np.ascontiguousarray(np.asarray(inputs["bk"], f32).reshape(8, 128).T),
            np.ascontiguousarray(
                (np.asarray(inputs["b1"], f32) + be1f @ W1f).reshape(32, 128).T
            ),
        ],
        axis=1,
    ).astype(f32)
    identb = np.concatenate(
        [
            np.eye(128, dtype=f32),
            np.concatenate([np.ones((1, 64), f32), np.zeros((1, 64), f32)], axis=1),
        ],
        axis=0,
    )

    shared = {"wblob": wb, "fblob": fblob, "identb": identb}

    in_maps = []
    for c in range(NCORES):
        b, qh = c // 2, c % 2
        q0 = qh * SQ
        if qh == 0:
            perm = np.arange(S)
        else:
            perm = np.r_[np.arange(512, 1024), np.arange(0, 512)]
        m = dict(shared)
        srcT = np.ascontiguousarray(src[b][perm].T).astype(bf16)
        dT = np.ascontiguousarray(distances[b, q0 : q0 + SQ][:, perm].T)
        ptb = np.exp(np.log(dT + np.float32(1e-9)) * np.float32(-nabs)).astype(bf16)
        m["srcp"] = np.ascontiguousarray(np.concatenate([srcT, ptb], axis=1))
        m["src_q"] = np.ascontiguousarray(
            src[b, q0 : q0 + SQ] + np.asarray(inputs["bo"], f32)[None, :]
        )
        in_maps.append(m)

    trace = bool(int(os.environ.get("BASS_KERNEL_TRACE", "0")))
    res = run_bass_kernel_spmd(
        nc,
        in_maps,
        core_ids=list(range(NCORES)),
        trace=trace,
        stitch_traces=False,
    )
    _CACHE["last_result"] = res

    out = np.empty((B, S, D), f32)
    for c in range(NCORES):
        b, qh = c // 2, c % 2
        out[b, qh * SQ : qh * SQ + SQ] = res.results[c]["out"]
    return out
